# revision 11
# baseline (speedup 1.0000x reference)
"""Trainium2 Bass kernel for DenseLayerWithComplexNeurons.

Reference computation (B=8, S=1024, DIN=1024, DOUT=1024, A=4, T=4, H=8):
    z = x @ W.T + bias                      # (B,S, A*DOUT)
    z -> (B,S,T,G,A), G = DOUT//T = 256
    h = tanh(z @ cw1[t] + cb1[t])           # (B,S,T,G,H)
    o = h @ cw2[t] + cb2[t]                 # (B,S,T,G) -> (B,S,DOUT)

Sharding: 8 cores = 4 token blocks (2048 tokens each) x 2 feature halves
(2048 W-rows / 512 neurons each).  All compute runs in a transposed layout
(features on partitions, tokens on the free dim) so the tiny per-neuron
MLPs become small constant matmuls on the tensor engine:
  - expansion E[t]: (g,a) -> (g,h) block-diagonal with cw1
  - reduction S[t]: (g,h) -> (g)   block-diagonal with cw2
The linear bias and cb1 are folded through cw1 into a single per-feature
bias bb added by the scalar engine inside tanh.  All matmul operands are
fp16 (same PE column rate as bf16, half the DMA traffic).

v2 over the first working version:
  - expansion pairs and reduction waves are interleaved between the main
    GEMM matmuls (inside the k loop) so their LDWEIGHTS hide in the
    previous matmul's drain window instead of stalling at cluster
    boundaries
  - reduction wave 0 uses start=True per column-tile (each tile overwrites
    its own full PSUM rows), removing the DVE pre-clear memsets
  - far fewer, larger DMAs (contiguous multi-fc / multi-k slices) spread
    over the sync + scalar HWDGE rings: less issue time, fewer semaphores
    to drain in the end-of-kernel barrier
  - warmup matmuls read garbage SBUF (no memset/cast dependency) so the
    PE starts ramping the HAM clock gate as soon as the engine is up
"""

import numpy as np

import concourse.bass as bass  # noqa: F401  (bass types via bacc)
import concourse.mybir as mybir
import concourse.tile as tile
from concourse import bacc
from concourse.bass_utils import run_bass_kernel_spmd

F32 = mybir.dt.float32
F16 = mybir.dt.float16

B, S, DIN, DOUT, A, T, H = 8, 1024, 1024, 1024, 4, 4, 8
G = DOUT // T                     # 256 neurons per cell type
NTOK = B * S                      # 8192 tokens
DP, TP = 4, 2                     # token blocks x feature halves
TOK_C = NTOK // DP                # 2048 tokens per core
NRN_C = DOUT // TP                # 512 neurons per core
FEAT_C = A * NRN_C                # 2048 A-expanded features per core
KC = DIN // 128                   # 8 contraction chunks
NB = TOK_C // 512                 # 4 token sub-blocks per core
FC = FEAT_C // 128                # 16 feature chunks per core
TL = FC // 2                      # 8 feature chunks per cell type

ds = bass.ds

_NC_CACHE = []


def _build_nc():
    nc = bacc.Bacc("TRN2", target_bir_lowering=False, debug=False, num_devices=8)

    # layouts chosen so every DMA slice is contiguous per partition:
    # xT[p, nb, k*512 + j] and wT[p, fc, k*128 + f]
    xT = nc.declare_dram_parameter("xT", [128, NB, KC * 512], F16,
                                   isOutput=False)
    wT = nc.declare_dram_parameter("wT", [128, FC, KC * 128], F16,
                                   isOutput=False)
    # esM[:, 0:2, :]  = expansion E (per tl), rows doubled (64+64)
    # esM[:, 2+tl*8+rr, :] = reduction S chunk rr (per tl)
    esM = nc.declare_dram_parameter("esM", [128, 18, 128], F16, isOutput=False)
    # bc[:, 0:32] = bb tanh bias, bc[:, 32+tl] = cb2
    bc = nc.declare_dram_parameter("bc", [128, 34], F32, isOutput=False)
    oT = nc.declare_dram_parameter("oT", [NRN_C, TOK_C], F32, isOutput=True)

    with tile.TileContext(nc) as tc:
        with tc.tile_pool(name="wp", bufs=1) as wp, \
             tc.tile_pool(name="cst", bufs=1) as cst, \
             tc.tile_pool(name="xp", bufs=4) as xp, \
             tc.tile_pool(name="zb", bufs=5) as zb, \
             tc.tile_pool(name="tb", bufs=14) as tb, \
             tc.tile_pool(name="ob", bufs=2) as ob, \
             tc.tile_pool(name="zp", bufs=2, space="PSUM") as zp, \
             tc.tile_pool(name="hp", bufs=4, space="PSUM") as hp, \
             tc.tile_pool(name="op", bufs=2, space="PSUM") as op:

            # --- PE warm-up + ACT table preload; wu is filled by the (idle)
            # gpsimd engine so the warmup starts the moment the PE boots,
            # well before the first HWDGE transfer can land (~8us).
            wu = cst.tile([128, 512], F16, tag="wu")
            nc.gpsimd.memset(wu[:], 0.001)
            garb = cst.tile([128, 16], F32, tag="garb")
            nc.scalar.activation(garb[:, 0:8], wu[:, 0:8],
                                 mybir.ActivationFunctionType.Tanh)
            for _ in range(11):
                wu_ps = zp.tile([128, 512], F32, tag="z")
                nc.tensor.matmul(wu_ps[:], wu[:, 0:128], wu[:],
                                 start=True, stop=True)

            # --- inputs.  Few, large, contiguous DMAs, ALL on the sync HWDGE
            # ring: a single queue keeps the end-of-kernel semaphore
            # retirement short, and one ring's issue stream is plenty.
            x_tiles = [xp.tile([128, KC * 512], F16, tag="x", name=f"x_{nb}")
                       for nb in range(NB)]
            w_all = wp.tile([128, FC, KC * 128], F16, tag="w")
            es_sb = cst.tile([128, 18, 128], F16, tag="es")
            bc_sb = cst.tile([128, 34], F32, tag="bc")
            nc.sync.dma_start(x_tiles[0][:], xT[:, 0])
            nc.sync.dma_start(w_all[:, ds(0, 2)], wT[:, ds(0, 2)])
            nc.sync.dma_start(w_all[:, ds(2, 2)], wT[:, ds(2, 2)])
            nc.sync.dma_start(es_sb[:], esM[:])
            nc.sync.dma_start(bc_sb[:], bc[:])
            nc.sync.dma_start(w_all[:, ds(4, 4)], wT[:, ds(4, 4)])
            nc.sync.dma_start(w_all[:, ds(8, 4)], wT[:, ds(8, 4)])
            nc.sync.dma_start(w_all[:, ds(12, 4)], wT[:, ds(12, 4)])
            nc.sync.dma_start(x_tiles[1][:], xT[:, 1])
            nc.sync.dma_start(x_tiles[2][:], xT[:, 2])
            nc.sync.dma_start(x_tiles[3][:], xT[:, 3])

            # --- software pipeline state.  Expansion pairs and reduction
            # waves are emitted in CLUSTERS ([pair, pair, wave] at k=3 of
            # every even slot): transitions into/out of partial-array slots
            # pay an exposed LDWEIGHTS (~100ns) each, so grouping them
            # amortizes that cost over more matmuls.
            e_stage = []                 # (slot, nb, grp, q, tl, z_sb)
            unit_ths = {}                # (nb, grp) -> [th] * 8
            red_pend = []                # [nb, grp, tl, ths, o_ps, wave]

            def emit_exp_pair(item):
                _, nb, grp, q, tl, z_sb = item
                fc = grp * 4 + q
                ths = unit_ths.setdefault((nb, grp), [])
                for half in range(2):
                    ci = fc * 2 + half
                    h_ps = hp.tile([128, 512], F32, tag="h")
                    nc.tensor.matmul(
                        h_ps[:],
                        es_sb[ds(half * 64, 64), tl, :],
                        z_sb[ds(half * 64, 64), :],
                        start=True, stop=True)
                    th = tb.tile([128, 512], F16, tag="t")
                    nc.scalar.activation(
                        th[:], h_ps[:],
                        mybir.ActivationFunctionType.Tanh,
                        bias=bc_sb[:, ds(ci, 1)])
                    ths.append(th)
                if q == 3:
                    red_pend.append([nb, grp, tl, unit_ths.pop((nb, grp)),
                                     None, 0])

            def emit_red_wave(unit):
                nb, grp, tl, ths, o_ps, wave = unit
                if wave == 0:
                    o_ps = op.tile([128, 512], F32, tag="o")
                    unit[4] = o_ps
                for rr in ((0, 2, 4, 6) if wave == 0 else (1, 3, 5, 7)):
                    j = rr // 2
                    nc.tensor.matmul(
                        o_ps[ds(32 * j, 32), :],
                        es_sb[:, 2 + tl * 8 + rr, ds(32 * j, 32)],
                        ths[rr][:],
                        start=(wave == 0), stop=(wave == 1 and rr == 7),
                        skip_group_check=True,
                        tile_position=(0, 32 * j))
                unit[5] += 1
                if wave == 1:
                    o_sb = ob.tile([128, 512], F32, tag="o")
                    nc.vector.tensor_scalar_add(
                        o_sb[:], o_ps[:], bc_sb[:, ds(32 + tl, 1)])
                    nc.sync.dma_start(
                        oT[ds(grp * 128, 128), ds(nb * 512, 512)], o_sb[:])
                    red_pend.pop(0)

            def cluster_hook(slot):
                # two pairs (for slots slot-2, slot-1), then one wave
                n_pairs = 0
                while (e_stage and e_stage[0][0] <= slot - 1
                       and n_pairs < 2):
                    emit_exp_pair(e_stage.pop(0))
                    n_pairs += 1
                # one reduction wave if a unit is ready (pairs done 2+ slots
                # ago so the tanh queue has drained)
                if red_pend:
                    unit = red_pend[0]
                    done_slot = 4 * (unit[0] * 4 + unit[1]) + 4
                    if slot >= done_slot + 2:
                        emit_red_wave(unit)

            slot = 0
            for nb in range(NB):
                x_nb = x_tiles[nb]
                for grp in range(4):
                    tl = grp // 2
                    for q in range(4):
                        fc = grp * 4 + q
                        z_ps = zp.tile([128, 512], F32, tag="z")
                        for k in range(KC):
                            nc.tensor.matmul(
                                z_ps[:],
                                w_all[:, fc, ds(k * 128, 128)],
                                x_nb[:, ds(k * 512, 512)],
                                start=(k == 0), stop=(k == KC - 1))
                            if k == 3 and slot % 2 == 0:
                                cluster_hook(slot)
                        z_sb = zb.tile([128, 512], F16, tag="z")
                        nc.vector.tensor_copy(z_sb[:], z_ps[:])
                        e_stage.append((slot, nb, grp, q, tl, z_sb))
                        slot += 1

            # --- tail flush: remaining pairs, then remaining waves
            while e_stage:
                emit_exp_pair(e_stage.pop(0))
            while red_pend:
                emit_red_wave(red_pend[0])

    nc.compile()
    return nc


def _host_prep(x, weight, bias, cw1, cb1, cw2, cb2):
    """Build the 8 per-core input maps (all host-side numpy)."""
    x2 = np.ascontiguousarray(x, dtype=np.float32).reshape(NTOK, DIN)
    weight = np.asarray(weight, dtype=np.float32)
    bias = np.asarray(bias, dtype=np.float32)
    cw1 = np.asarray(cw1, dtype=np.float32)   # (T, A, H)
    cb1 = np.asarray(cb1, dtype=np.float32)   # (T, H)
    cw2 = np.asarray(cw2, dtype=np.float32)   # (T, H)
    cb2 = np.asarray(cb2, dtype=np.float32)   # (T,)

    # xT[p, nb, k*512 + j] = x2[tok0 + nb*512 + j, k*128 + p]
    xT_all = []
    for i in range(DP):
        blk = x2[i * TOK_C:(i + 1) * TOK_C]            # (TOK_C, DIN)
        t = blk.T.reshape(KC, 128, NB, 512)            # (k, p, nb, j)
        t = t.transpose(1, 2, 0, 3).reshape(128, NB, KC * 512)
        xT_all.append(np.ascontiguousarray(t, dtype=np.float16))

    # wT[p, fc, k*128 + f] = W[j*FEAT_C + fc*128 + f, k*128 + p]
    wT_all = []
    for j in range(TP):
        wj = weight[j * FEAT_C:(j + 1) * FEAT_C]       # (FEAT_C, DIN)
        t = wj.T.reshape(KC, 128, FC, 128)             # (k, p, fc, f)
        t = t.transpose(1, 2, 0, 3).reshape(128, FC, KC * 128)
        wT_all.append(np.ascontiguousarray(t, dtype=np.float16))

    # esM: E[t] (g*4+a, g16*8+h) block-diag cw1 (rows doubled);
    #      S[t] chunk rr: (g*8+h, rr*16+g) block-diag cw2
    es_all, bc_all = [], []
    for j in range(TP):
        esj = np.zeros((128, 18, 128), np.float32)
        for tl in range(2):
            t = 2 * j + tl
            for g16 in range(16):
                for a in range(A):
                    for h in range(H):
                        v = cw1[t, a, h]
                        esj[g16 * 4 + a, tl, g16 * 8 + h] = v
                        esj[64 + g16 * 4 + a, tl, g16 * 8 + h] = v
            for rr in range(8):
                for g in range(16):
                    for h in range(H):
                        esj[g * 8 + h, 2 + tl * 8 + rr, rr * 16 + g] = cw2[t, h]
        es_all.append(esj.astype(np.float16))

        # bb[f2=(n_loc, h)] = sum_a cw1[t,a,h]*bias[t*1024+g_t*4+a] + cb1[t,h]
        nl = np.arange(NRN_C)
        t_of = (j * NRN_C + nl) // G                   # cell type per neuron
        gt = (j * NRN_C + nl) % G                      # group within type
        bias_ga = bias.reshape(T, G, A)[t_of, gt]      # (NRN_C, A)
        bbv = np.einsum('na,nah->nh', bias_ga, cw1[t_of]) + cb1[t_of]
        bcj = np.zeros((128, 34), np.float32)
        bcj[:, 0:32] = bbv.reshape(NRN_C * H).reshape(2 * FC, 128).T
        for tl in range(2):
            bcj[:, 32 + tl] = cb2[2 * j + tl]
        bc_all.append(bcj)

    wum = np.full((128, 512), 0.001, np.float16)
    in_maps = []
    for c in range(8):
        i, j = c // TP, c % TP
        in_maps.append({
            "xT": xT_all[i], "wT": wT_all[j],
            "esM": es_all[j], "bc": bc_all[j], "wuM": wum,
        })
    return in_maps


def kernel(x, weight, bias, cw1, cb1, cw2, cb2):
    in_maps = _host_prep(x, weight, bias, cw1, cb1, cw2, cb2)
    if not _NC_CACHE:
        _NC_CACHE.append(_build_nc())
    nc = _NC_CACHE[0]
    try:
        res = run_bass_kernel_spmd(nc, in_maps, list(range(8)))
    except Exception:
        # transient NRT device faults have been observed once after crashed
        # runs; a clean retry in the same process recovers
        res = run_bass_kernel_spmd(nc, in_maps, list(range(8)))
    out = np.empty((NTOK, DOUT), np.float32)
    for c in range(8):
        i, j = c // TP, c % TP
        oc = res.results[c]["oT"]                      # (NRN_C, TOK_C)
        out[i * TOK_C:(i + 1) * TOK_C, j * NRN_C:(j + 1) * NRN_C] = oc.T
    return out.reshape(B, S, DOUT)


# revision 17
# speedup vs baseline: 1.1803x; 1.1803x over previous
"""Trainium2 Bass kernel for DenseLayerWithComplexNeurons.

Reference computation (B=8, S=1024, DIN=1024, DOUT=1024, A=4, T=4, H=8):
    z = x @ W.T + bias                      # (B,S, A*DOUT)
    z -> (B,S,T,G,A), G = DOUT//T = 256
    h = tanh(z @ cw1[t] + cb1[t])           # (B,S,T,G,H)
    o = h @ cw2[t] + cb2[t]                 # (B,S,T,G) -> (B,S,DOUT)

Sharding: 8 cores = 4 token blocks (2048 tokens each) x 2 feature halves
(2048 W-rows / 512 neurons each).  All compute runs in a transposed layout
(features on partitions, tokens on the free dim) so the tiny per-neuron
MLPs become small constant matmuls on the tensor engine:
  - expansion E[t]: (g,a) -> (g,h) block-diagonal with cw1
  - reduction S[t]: (g,h) -> (g)   block-diagonal with cw2
The linear bias and cb1 are folded through cw1 into a single per-feature
bias bb added by the scalar engine inside tanh.  All matmul operands are
fp16 (same PE column rate as bf16, half the DMA traffic).

v2 over the first working version:
  - expansion pairs and reduction waves are interleaved between the main
    GEMM matmuls (inside the k loop) so their LDWEIGHTS hide in the
    previous matmul's drain window instead of stalling at cluster
    boundaries
  - reduction wave 0 uses start=True per column-tile (each tile overwrites
    its own full PSUM rows), removing the DVE pre-clear memsets
  - far fewer, larger DMAs (contiguous multi-fc / multi-k slices) spread
    over the sync + scalar HWDGE rings: less issue time, fewer semaphores
    to drain in the end-of-kernel barrier
  - warmup matmuls read garbage SBUF (no memset/cast dependency) so the
    PE starts ramping the HAM clock gate as soon as the engine is up
"""

import numpy as np

import concourse.bass as bass  # noqa: F401  (bass types via bacc)
import concourse.mybir as mybir
import concourse.tile as tile
from concourse import bacc
from concourse.bass_utils import run_bass_kernel_spmd

F32 = mybir.dt.float32
F16 = mybir.dt.float16

B, S, DIN, DOUT, A, T, H = 8, 1024, 1024, 1024, 4, 4, 8
G = DOUT // T                     # 256 neurons per cell type
NTOK = B * S                      # 8192 tokens
DP, TP = 4, 2                     # token blocks x feature halves
TOK_C = NTOK // DP                # 2048 tokens per core
NRN_C = DOUT // TP                # 512 neurons per core
FEAT_C = A * NRN_C                # 2048 A-expanded features per core
KC = DIN // 128                   # 8 contraction chunks
NB = TOK_C // 512                 # 4 token sub-blocks per core
FC = FEAT_C // 128                # 16 feature chunks per core
TL = FC // 2                      # 8 feature chunks per cell type

ds = bass.ds

_NC_CACHE = []


def _build_nc():
    nc = bacc.Bacc("TRN2", target_bir_lowering=False, debug=False, num_devices=8)

    # layouts chosen so every DMA slice is contiguous per partition:
    # xT[p, nb, k*512 + j] and wT[p, fc, k*128 + f]
    xT = nc.declare_dram_parameter("xT", [128, NB, KC * 512], F16,
                                   isOutput=False)
    wT = nc.declare_dram_parameter("wT", [128, FC, KC * 128], F16,
                                   isOutput=False)
    # esM[:, 0:2, :]  = expansion E (per tl), rows doubled (64+64)
    # esM[:, 2+tl*8+rr, :] = reduction S chunk rr (per tl)
    esM = nc.declare_dram_parameter("esM", [128, 18, 128], F16, isOutput=False)
    # bc[:, 0:32] = bb tanh bias, bc[:, 32+tl] = cb2
    bc = nc.declare_dram_parameter("bc", [128, 34], F32, isOutput=False)
    oT = nc.declare_dram_parameter("oT", [NRN_C, TOK_C], F32, isOutput=True)

    with tile.TileContext(nc) as tc:
        with tc.tile_pool(name="wp", bufs=1) as wp, \
             tc.tile_pool(name="cst", bufs=1) as cst, \
             tc.tile_pool(name="xp", bufs=4) as xp, \
             tc.tile_pool(name="zb", bufs=5) as zb, \
             tc.tile_pool(name="tb", bufs=14) as tb, \
             tc.tile_pool(name="ob", bufs=2) as ob, \
             tc.tile_pool(name="zp", bufs=2, space="PSUM") as zp, \
             tc.tile_pool(name="hp", bufs=4, space="PSUM") as hp, \
             tc.tile_pool(name="op", bufs=2, space="PSUM") as op:

            # --- PE warm-up + ACT table preload; wu is filled by the (idle)
            # gpsimd engine so the warmup starts the moment the PE boots,
            # well before the first HWDGE transfer can land (~8us).
            wu = cst.tile([128, 512], F16, tag="wu")
            nc.gpsimd.memset(wu[:], 0.001)
            garb = cst.tile([128, 16], F32, tag="garb")
            nc.scalar.activation(garb[:, 0:8], wu[:, 0:8],
                                 mybir.ActivationFunctionType.Tanh)
            for _ in range(10):
                wu_ps = zp.tile([128, 512], F32, tag="z")
                nc.tensor.matmul(wu_ps[:], wu[:, 0:128], wu[:],
                                 start=True, stop=True)

            # --- inputs.  Few, large, contiguous DMAs, ALL on the sync HWDGE
            # ring: a single queue keeps the end-of-kernel semaphore
            # retirement short, and one ring's issue stream is plenty.
            x_tiles = [xp.tile([128, KC * 512], F16, tag="x", name=f"x_{nb}")
                       for nb in range(NB)]
            w_all = wp.tile([128, FC, KC * 128], F16, tag="w")
            es_sb = cst.tile([128, 18, 128], F16, tag="es")
            bc_sb = cst.tile([128, 34], F32, tag="bc")
            nc.sync.dma_start(x_tiles[0][:], xT[:, 0])
            nc.sync.dma_start(w_all[:, ds(0, 2)], wT[:, ds(0, 2)])
            nc.sync.dma_start(w_all[:, ds(2, 2)], wT[:, ds(2, 2)])
            nc.sync.dma_start(es_sb[:], esM[:])
            nc.sync.dma_start(bc_sb[:], bc[:])
            nc.sync.dma_start(w_all[:, ds(4, 4)], wT[:, ds(4, 4)])
            nc.sync.dma_start(w_all[:, ds(8, 4)], wT[:, ds(8, 4)])
            nc.sync.dma_start(w_all[:, ds(12, 4)], wT[:, ds(12, 4)])
            nc.sync.dma_start(x_tiles[1][:], xT[:, 1])
            nc.sync.dma_start(x_tiles[2][:], xT[:, 2])
            nc.sync.dma_start(x_tiles[3][:], xT[:, 3])

            # --- software pipeline state.  Expansion pairs and reduction
            # waves are emitted in CLUSTERS ([pair, pair, wave] at k=3 of
            # every even slot): transitions into/out of partial-array slots
            # pay an exposed LDWEIGHTS (~100ns) each, so grouping them
            # amortizes that cost over more matmuls.
            e_stage = []                 # (slot, nb, grp, q, tl, z_sb)
            unit_ths = {}                # (nb, grp) -> [th] * 8
            red_pend = []                # [nb, grp, tl, ths, o_ps, wave]

            def emit_exp_half(item, half):
                _, nb, grp, q, tl, z_sb = item
                fc = grp * 4 + q
                ths = unit_ths.setdefault((nb, grp), [None] * 8)
                ci = fc * 2 + half
                h_ps = hp.tile([128, 512], F32, tag="h")
                nc.tensor.matmul(
                    h_ps[:],
                    es_sb[ds(half * 64, 64), tl, :],
                    z_sb[ds(half * 64, 64), :],
                    start=True, stop=True)
                th = tb.tile([128, 512], F16, tag="t")
                nc.scalar.activation(
                    th[:], h_ps[:],
                    mybir.ActivationFunctionType.Tanh,
                    bias=bc_sb[:, ds(ci, 1)])
                ths[2 * q + half] = th

            def emit_exp_pair(item):
                emit_exp_half(item, 0)
                emit_exp_half(item, 1)
                _, nb, grp, q, tl, _ = item
                if q == 3:
                    red_pend.append([nb, grp, tl, unit_ths.pop((nb, grp)),
                                     None, 0])

            def emit_red_wave(unit):
                nb, grp, tl, ths, o_ps, wave = unit
                if wave == 0:
                    o_ps = op.tile([128, 512], F32, tag="o")
                    unit[4] = o_ps
                for rr in ((0, 2, 4, 6) if wave == 0 else (1, 3, 5, 7)):
                    j = rr // 2
                    nc.tensor.matmul(
                        o_ps[ds(32 * j, 32), :],
                        es_sb[:, 2 + tl * 8 + rr, ds(32 * j, 32)],
                        ths[rr][:],
                        start=(wave == 0), stop=(wave == 1 and rr == 7),
                        skip_group_check=True,
                        tile_position=(0, 32 * j))
                unit[5] += 1
                if wave == 1:
                    o_sb = ob.tile([128, 512], F32, tag="o")
                    dst = oT[ds(grp * 128, 128), ds(nb * 512, 512)]
                    if nb == NB - 1 and grp == 3:
                        # tail: halve the add+DMA chain across two rings so
                        # the first half's transfer overlaps the second add
                        for hh in range(2):
                            sl = ds(hh * 256, 256)
                            nc.vector.tensor_scalar_add(
                                o_sb[:, sl], o_ps[:, sl],
                                bc_sb[:, ds(32 + tl, 1)])
                            ring = nc.sync if hh == 0 else nc.scalar
                            ring.dma_start(
                                oT[ds(grp * 128, 128),
                                   ds(nb * 512 + hh * 256, 256)],
                                o_sb[:, sl])
                    else:
                        nc.vector.tensor_scalar_add(
                            o_sb[:], o_ps[:], bc_sb[:, ds(32 + tl, 1)])
                        nc.sync.dma_start(dst, o_sb[:])
                    red_pend.pop(0)

            def cluster_hook(slot):
                # two pairs (for slots slot-2, slot-1), then one wave
                n_pairs = 0
                while (e_stage and e_stage[0][0] <= slot - 1
                       and n_pairs < 2):
                    emit_exp_pair(e_stage.pop(0))
                    n_pairs += 1
                # one reduction wave if a unit is ready (pairs done 2+ slots
                # ago so the tanh queue has drained)
                if red_pend:
                    unit = red_pend[0]
                    done_slot = 4 * (unit[0] * 4 + unit[1]) + 4
                    if slot >= done_slot + 2:
                        emit_red_wave(unit)

            slot = 0
            for nb in range(NB):
                x_nb = x_tiles[nb]
                for grp in range(4):
                    tl = grp // 2
                    for q in range(4):
                        fc = grp * 4 + q
                        z_ps = zp.tile([128, 512], F32, tag="z")
                        for k in range(KC):
                            nc.tensor.matmul(
                                z_ps[:],
                                w_all[:, fc, ds(k * 128, 128)],
                                x_nb[:, ds(k * 512, 512)],
                                start=(k == 0), stop=(k == KC - 1))
                            if k == 3 and (slot % 2 == 0 or slot == 63):
                                cluster_hook(slot)
                        z_sb = zb.tile([128, 512], F16, tag="z")
                        nc.vector.tensor_copy(z_sb[:], z_ps[:])
                        e_stage.append((slot, nb, grp, q, tl, z_sb))
                        slot += 1

            # --- tail flush: remaining pairs, then remaining waves (wave0
            # only waits on the half-0 tanh and overlaps the half-1 one)
            while e_stage:
                emit_exp_pair(e_stage.pop(0))
            while red_pend:
                emit_red_wave(red_pend[0])

    nc.compile()
    return nc


def _host_prep(x, weight, bias, cw1, cb1, cw2, cb2):
    """Build the 8 per-core input maps (all host-side numpy)."""
    x2 = np.ascontiguousarray(x, dtype=np.float32).reshape(NTOK, DIN)
    weight = np.asarray(weight, dtype=np.float32)
    bias = np.asarray(bias, dtype=np.float32)
    cw1 = np.asarray(cw1, dtype=np.float32)   # (T, A, H)
    cb1 = np.asarray(cb1, dtype=np.float32)   # (T, H)
    cw2 = np.asarray(cw2, dtype=np.float32)   # (T, H)
    cb2 = np.asarray(cb2, dtype=np.float32)   # (T,)

    # xT[p, nb, k*512 + j] = x2[tok0 + nb*512 + j, k*128 + p]
    xT_all = []
    for i in range(DP):
        blk = x2[i * TOK_C:(i + 1) * TOK_C]            # (TOK_C, DIN)
        t = blk.T.reshape(KC, 128, NB, 512)            # (k, p, nb, j)
        t = t.transpose(1, 2, 0, 3).reshape(128, NB, KC * 512)
        xT_all.append(np.ascontiguousarray(t, dtype=np.float16))

    # wT[p, fc, k*128 + f] = W[j*FEAT_C + fc*128 + f, k*128 + p]
    wT_all = []
    for j in range(TP):
        wj = weight[j * FEAT_C:(j + 1) * FEAT_C]       # (FEAT_C, DIN)
        t = wj.T.reshape(KC, 128, FC, 128)             # (k, p, fc, f)
        t = t.transpose(1, 2, 0, 3).reshape(128, FC, KC * 128)
        wT_all.append(np.ascontiguousarray(t, dtype=np.float16))

    # esM: E[t] (g*4+a, g16*8+h) block-diag cw1 (rows doubled);
    #      S[t] chunk rr: (g*8+h, rr*16+g) block-diag cw2
    es_all, bc_all = [], []
    for j in range(TP):
        esj = np.zeros((128, 18, 128), np.float32)
        for tl in range(2):
            t = 2 * j + tl
            for g16 in range(16):
                for a in range(A):
                    for h in range(H):
                        v = cw1[t, a, h]
                        esj[g16 * 4 + a, tl, g16 * 8 + h] = v
                        esj[64 + g16 * 4 + a, tl, g16 * 8 + h] = v
            for rr in range(8):
                for g in range(16):
                    for h in range(H):
                        esj[g * 8 + h, 2 + tl * 8 + rr, rr * 16 + g] = cw2[t, h]
        es_all.append(esj.astype(np.float16))

        # bb[f2=(n_loc, h)] = sum_a cw1[t,a,h]*bias[t*1024+g_t*4+a] + cb1[t,h]
        nl = np.arange(NRN_C)
        t_of = (j * NRN_C + nl) // G                   # cell type per neuron
        gt = (j * NRN_C + nl) % G                      # group within type
        bias_ga = bias.reshape(T, G, A)[t_of, gt]      # (NRN_C, A)
        bbv = np.einsum('na,nah->nh', bias_ga, cw1[t_of]) + cb1[t_of]
        bcj = np.zeros((128, 34), np.float32)
        bcj[:, 0:32] = bbv.reshape(NRN_C * H).reshape(2 * FC, 128).T
        for tl in range(2):
            bcj[:, 32 + tl] = cb2[2 * j + tl]
        bc_all.append(bcj)

    wum = np.full((128, 512), 0.001, np.float16)
    in_maps = []
    for c in range(8):
        i, j = c // TP, c % TP
        in_maps.append({
            "xT": xT_all[i], "wT": wT_all[j],
            "esM": es_all[j], "bc": bc_all[j], "wuM": wum,
        })
    return in_maps


def kernel(x, weight, bias, cw1, cb1, cw2, cb2):
    in_maps = _host_prep(x, weight, bias, cw1, cb1, cw2, cb2)
    if not _NC_CACHE:
        _NC_CACHE.append(_build_nc())
    nc = _NC_CACHE[0]
    try:
        res = run_bass_kernel_spmd(nc, in_maps, list(range(8)))
    except Exception:
        # transient NRT device faults have been observed once after crashed
        # runs; a clean retry in the same process recovers
        res = run_bass_kernel_spmd(nc, in_maps, list(range(8)))
    out = np.empty((NTOK, DOUT), np.float32)
    for c in range(8):
        i, j = c // TP, c % TP
        oc = res.results[c]["oT"]                      # (NRN_C, TOK_C)
        out[i * TOK_C:(i + 1) * TOK_C, j * NRN_C:(j + 1) * NRN_C] = oc.T
    return out.reshape(B, S, DOUT)


# revision 18
# speedup vs baseline: 1.1815x; 1.0010x over previous
"""Trainium2 Bass kernel for DenseLayerWithComplexNeurons.

Reference computation (B=8, S=1024, DIN=1024, DOUT=1024, A=4, T=4, H=8):
    z = x @ W.T + bias                      # (B,S, A*DOUT)
    z -> (B,S,T,G,A), G = DOUT//T = 256
    h = tanh(z @ cw1[t] + cb1[t])           # (B,S,T,G,H)
    o = h @ cw2[t] + cb2[t]                 # (B,S,T,G) -> (B,S,DOUT)

Sharding: 8 cores = 4 token blocks (2048 tokens each) x 2 feature halves
(2048 W-rows / 512 neurons each).  All compute runs in a transposed layout
(features on partitions, tokens on the free dim) so the tiny per-neuron
MLPs become small constant matmuls on the tensor engine:
  - expansion E[t]: (g,a) -> (g,h) block-diagonal with cw1
  - reduction S[t]: (g,h) -> (g)   block-diagonal with cw2
The linear bias and cb1 are folded through cw1 into a single per-feature
bias bb added by the scalar engine inside tanh.  All matmul operands are
fp16 (same PE column rate as bf16, half the DMA traffic).

v2 over the first working version:
  - expansion pairs and reduction waves are interleaved between the main
    GEMM matmuls (inside the k loop) so their LDWEIGHTS hide in the
    previous matmul's drain window instead of stalling at cluster
    boundaries
  - reduction wave 0 uses start=True per column-tile (each tile overwrites
    its own full PSUM rows), removing the DVE pre-clear memsets
  - far fewer, larger DMAs (contiguous multi-fc / multi-k slices) spread
    over the sync + scalar HWDGE rings: less issue time, fewer semaphores
    to drain in the end-of-kernel barrier
  - warmup matmuls read garbage SBUF (no memset/cast dependency) so the
    PE starts ramping the HAM clock gate as soon as the engine is up
"""

import numpy as np

import concourse.bass as bass  # noqa: F401  (bass types via bacc)
import concourse.mybir as mybir
import concourse.tile as tile
from concourse import bacc
from concourse.bass_utils import run_bass_kernel_spmd

F32 = mybir.dt.float32
F16 = mybir.dt.float16

B, S, DIN, DOUT, A, T, H = 8, 1024, 1024, 1024, 4, 4, 8
G = DOUT // T                     # 256 neurons per cell type
NTOK = B * S                      # 8192 tokens
DP, TP = 4, 2                     # token blocks x feature halves
TOK_C = NTOK // DP                # 2048 tokens per core
NRN_C = DOUT // TP                # 512 neurons per core
FEAT_C = A * NRN_C                # 2048 A-expanded features per core
KC = DIN // 128                   # 8 contraction chunks
NB = TOK_C // 512                 # 4 token sub-blocks per core
FC = FEAT_C // 128                # 16 feature chunks per core
TL = FC // 2                      # 8 feature chunks per cell type

ds = bass.ds

_NC_CACHE = []


def _build_nc():
    nc = bacc.Bacc("TRN2", target_bir_lowering=False, debug=False, num_devices=8)

    # layouts chosen so every DMA slice is contiguous per partition:
    # xT[p, nb, k*512 + j] and wT[p, fc, k*128 + f]
    xT = nc.declare_dram_parameter("xT", [128, NB, KC * 512], F16,
                                   isOutput=False)
    wT = nc.declare_dram_parameter("wT", [128, FC, KC * 128], F16,
                                   isOutput=False)
    # esM[:, 0:2, :]  = expansion E (per tl), rows doubled (64+64)
    # esM[:, 2+tl*8+rr, :] = reduction S chunk rr (per tl)
    esM = nc.declare_dram_parameter("esM", [128, 18, 128], F16, isOutput=False)
    # bc[:, 0:32] = bb tanh bias, bc[:, 32+tl] = cb2
    bc = nc.declare_dram_parameter("bc", [128, 34], F32, isOutput=False)
    oT = nc.declare_dram_parameter("oT", [NRN_C, TOK_C], F32, isOutput=True)

    with tile.TileContext(nc) as tc:
        with tc.tile_pool(name="wp", bufs=1) as wp, \
             tc.tile_pool(name="cst", bufs=1) as cst, \
             tc.tile_pool(name="xp", bufs=4) as xp, \
             tc.tile_pool(name="zb", bufs=5) as zb, \
             tc.tile_pool(name="tb", bufs=14) as tb, \
             tc.tile_pool(name="ob", bufs=2) as ob, \
             tc.tile_pool(name="zp", bufs=2, space="PSUM") as zp, \
             tc.tile_pool(name="hp", bufs=5, space="PSUM") as hp, \
             tc.tile_pool(name="op", bufs=1, space="PSUM") as op:

            # --- PE warm-up + ACT table preload; wu is filled by the (idle)
            # gpsimd engine so the warmup starts the moment the PE boots,
            # well before the first HWDGE transfer can land (~8us).
            wu = cst.tile([128, 512], F16, tag="wu")
            nc.gpsimd.memset(wu[:], 0.001)
            garb = cst.tile([128, 16], F32, tag="garb")
            nc.scalar.activation(garb[:, 0:8], wu[:, 0:8],
                                 mybir.ActivationFunctionType.Tanh)
            for _ in range(11):
                wu_ps = zp.tile([128, 512], F32, tag="z")
                nc.tensor.matmul(wu_ps[:], wu[:, 0:128], wu[:],
                                 start=True, stop=True)

            # --- inputs.  Few, large, contiguous DMAs, ALL on the sync HWDGE
            # ring: a single queue keeps the end-of-kernel semaphore
            # retirement short, and one ring's issue stream is plenty.
            x_tiles = [xp.tile([128, KC * 512], F16, tag="x", name=f"x_{nb}")
                       for nb in range(NB)]
            w_all = wp.tile([128, FC, KC * 128], F16, tag="w")
            es_sb = cst.tile([128, 18, 128], F16, tag="es")
            bc_sb = cst.tile([128, 34], F32, tag="bc")
            nc.sync.dma_start(w_all[:, ds(0, 1)], wT[:, ds(0, 1)])
            nc.sync.dma_start(x_tiles[0][:], xT[:, 0])
            nc.sync.dma_start(w_all[:, ds(1, 1)], wT[:, ds(1, 1)])
            nc.sync.dma_start(w_all[:, ds(2, 2)], wT[:, ds(2, 2)])
            nc.sync.dma_start(es_sb[:], esM[:])
            nc.sync.dma_start(bc_sb[:], bc[:])
            nc.sync.dma_start(w_all[:, ds(4, 4)], wT[:, ds(4, 4)])
            nc.sync.dma_start(w_all[:, ds(8, 4)], wT[:, ds(8, 4)])
            nc.sync.dma_start(w_all[:, ds(12, 4)], wT[:, ds(12, 4)])
            nc.sync.dma_start(x_tiles[1][:], xT[:, 1])
            nc.sync.dma_start(x_tiles[2][:], xT[:, 2])
            nc.sync.dma_start(x_tiles[3][:], xT[:, 3])

            # --- software pipeline state.  Expansion pairs and reduction
            # waves are emitted in CLUSTERS ([pair, pair, wave] at k=3 of
            # every even slot): transitions into/out of partial-array slots
            # pay an exposed LDWEIGHTS (~100ns) each, so grouping them
            # amortizes that cost over more matmuls.
            e_stage = []                 # (slot, nb, grp, q, tl, z_sb)
            unit_ths = {}                # (nb, grp) -> [th] * 8
            red_pend = []                # [nb, grp, tl, ths, o_ps, wave]

            def emit_exp_half(item, half):
                _, nb, grp, q, tl, z_sb = item
                fc = grp * 4 + q
                ths = unit_ths.setdefault((nb, grp), [None] * 8)
                ci = fc * 2 + half
                h_ps = hp.tile([128, 512], F32, tag="h")
                nc.tensor.matmul(
                    h_ps[:],
                    es_sb[ds(half * 64, 64), tl, :],
                    z_sb[ds(half * 64, 64), :],
                    start=True, stop=True)
                th = tb.tile([128, 512], F16, tag="t")
                nc.scalar.activation(
                    th[:], h_ps[:],
                    mybir.ActivationFunctionType.Tanh,
                    bias=bc_sb[:, ds(ci, 1)])
                ths[2 * q + half] = th

            def emit_exp_pair(item):
                emit_exp_half(item, 0)
                emit_exp_half(item, 1)
                _, nb, grp, q, tl, _ = item
                if q == 3:
                    red_pend.append([nb, grp, tl, unit_ths.pop((nb, grp)),
                                     None, 0])

            def emit_red_wave(unit):
                nb, grp, tl, ths, o_ps, wave = unit
                if wave == 0:
                    o_ps = op.tile([128, 512], F32, tag="o")
                    unit[4] = o_ps
                for rr in ((0, 2, 4, 6) if wave == 0 else (1, 3, 5, 7)):
                    j = rr // 2
                    nc.tensor.matmul(
                        o_ps[ds(32 * j, 32), :],
                        es_sb[:, 2 + tl * 8 + rr, ds(32 * j, 32)],
                        ths[rr][:],
                        start=(wave == 0), stop=(wave == 1 and rr == 7),
                        skip_group_check=True,
                        tile_position=(0, 32 * j))
                unit[5] += 1
                if wave == 1:
                    o_sb = ob.tile([128, 512], F32, tag="o")
                    dst = oT[ds(grp * 128, 128), ds(nb * 512, 512)]
                    if nb == NB - 1 and grp == 3:
                        # tail: halve the add+DMA chain across two rings so
                        # the first half's transfer overlaps the second add
                        for hh in range(2):
                            sl = ds(hh * 256, 256)
                            nc.vector.tensor_scalar_add(
                                o_sb[:, sl], o_ps[:, sl],
                                bc_sb[:, ds(32 + tl, 1)])
                            ring = nc.sync if hh == 0 else nc.scalar
                            ring.dma_start(
                                oT[ds(grp * 128, 128),
                                   ds(nb * 512 + hh * 256, 256)],
                                o_sb[:, sl])
                    else:
                        nc.vector.tensor_scalar_add(
                            o_sb[:], o_ps[:], bc_sb[:, ds(32 + tl, 1)])
                        nc.sync.dma_start(dst, o_sb[:])
                    red_pend.pop(0)

            def cluster_hook(slot):
                # two pairs (for slots slot-2, slot-1), then one wave
                n_pairs = 0
                while (e_stage and e_stage[0][0] <= slot - 1
                       and n_pairs < 2):
                    emit_exp_pair(e_stage.pop(0))
                    n_pairs += 1
                # one reduction wave if a unit is ready (pairs done 2+ slots
                # ago so the tanh queue has drained)
                if red_pend:
                    unit = red_pend[0]
                    done_slot = 4 * (unit[0] * 4 + unit[1]) + 4
                    if slot >= done_slot + 2:
                        emit_red_wave(unit)

            slot = 0
            for nb in range(NB):
                x_nb = x_tiles[nb]
                for grp in range(4):
                    tl = grp // 2
                    for q in range(4):
                        fc = grp * 4 + q
                        z_ps = zp.tile([128, 512], F32, tag="z")
                        for k in range(KC):
                            nc.tensor.matmul(
                                z_ps[:],
                                w_all[:, fc, ds(k * 128, 128)],
                                x_nb[:, ds(k * 512, 512)],
                                start=(k == 0), stop=(k == KC - 1))
                            if k == 3 and (slot % 2 == 0 or slot == 63):
                                cluster_hook(slot)
                        z_sb = zb.tile([128, 512], F16, tag="z")
                        nc.vector.tensor_copy(z_sb[:], z_ps[:])
                        e_stage.append((slot, nb, grp, q, tl, z_sb))
                        slot += 1

            # --- tail flush: remaining pairs, then remaining waves (wave0
            # only waits on the half-0 tanh and overlaps the half-1 one)
            while e_stage:
                emit_exp_pair(e_stage.pop(0))
            while red_pend:
                emit_red_wave(red_pend[0])

    nc.compile()
    return nc


def _host_prep(x, weight, bias, cw1, cb1, cw2, cb2):
    """Build the 8 per-core input maps (all host-side numpy)."""
    x2 = np.ascontiguousarray(x, dtype=np.float32).reshape(NTOK, DIN)
    weight = np.asarray(weight, dtype=np.float32)
    bias = np.asarray(bias, dtype=np.float32)
    cw1 = np.asarray(cw1, dtype=np.float32)   # (T, A, H)
    cb1 = np.asarray(cb1, dtype=np.float32)   # (T, H)
    cw2 = np.asarray(cw2, dtype=np.float32)   # (T, H)
    cb2 = np.asarray(cb2, dtype=np.float32)   # (T,)

    # xT[p, nb, k*512 + j] = x2[tok0 + nb*512 + j, k*128 + p]
    xT_all = []
    for i in range(DP):
        blk = x2[i * TOK_C:(i + 1) * TOK_C]            # (TOK_C, DIN)
        t = blk.T.reshape(KC, 128, NB, 512)            # (k, p, nb, j)
        t = t.transpose(1, 2, 0, 3).reshape(128, NB, KC * 512)
        xT_all.append(np.ascontiguousarray(t, dtype=np.float16))

    # wT[p, fc, k*128 + f] = W[j*FEAT_C + fc*128 + f, k*128 + p]
    wT_all = []
    for j in range(TP):
        wj = weight[j * FEAT_C:(j + 1) * FEAT_C]       # (FEAT_C, DIN)
        t = wj.T.reshape(KC, 128, FC, 128)             # (k, p, fc, f)
        t = t.transpose(1, 2, 0, 3).reshape(128, FC, KC * 128)
        wT_all.append(np.ascontiguousarray(t, dtype=np.float16))

    # esM: E[t] (g*4+a, g16*8+h) block-diag cw1 (rows doubled);
    #      S[t] chunk rr: (g*8+h, rr*16+g) block-diag cw2
    es_all, bc_all = [], []
    for j in range(TP):
        esj = np.zeros((128, 18, 128), np.float32)
        for tl in range(2):
            t = 2 * j + tl
            for g16 in range(16):
                for a in range(A):
                    for h in range(H):
                        v = cw1[t, a, h]
                        esj[g16 * 4 + a, tl, g16 * 8 + h] = v
                        esj[64 + g16 * 4 + a, tl, g16 * 8 + h] = v
            for rr in range(8):
                for g in range(16):
                    for h in range(H):
                        esj[g * 8 + h, 2 + tl * 8 + rr, rr * 16 + g] = cw2[t, h]
        es_all.append(esj.astype(np.float16))

        # bb[f2=(n_loc, h)] = sum_a cw1[t,a,h]*bias[t*1024+g_t*4+a] + cb1[t,h]
        nl = np.arange(NRN_C)
        t_of = (j * NRN_C + nl) // G                   # cell type per neuron
        gt = (j * NRN_C + nl) % G                      # group within type
        bias_ga = bias.reshape(T, G, A)[t_of, gt]      # (NRN_C, A)
        bbv = np.einsum('na,nah->nh', bias_ga, cw1[t_of]) + cb1[t_of]
        bcj = np.zeros((128, 34), np.float32)
        bcj[:, 0:32] = bbv.reshape(NRN_C * H).reshape(2 * FC, 128).T
        for tl in range(2):
            bcj[:, 32 + tl] = cb2[2 * j + tl]
        bc_all.append(bcj)

    wum = np.full((128, 512), 0.001, np.float16)
    in_maps = []
    for c in range(8):
        i, j = c // TP, c % TP
        in_maps.append({
            "xT": xT_all[i], "wT": wT_all[j],
            "esM": es_all[j], "bc": bc_all[j], "wuM": wum,
        })
    return in_maps


def kernel(x, weight, bias, cw1, cb1, cw2, cb2):
    in_maps = _host_prep(x, weight, bias, cw1, cb1, cw2, cb2)
    if not _NC_CACHE:
        _NC_CACHE.append(_build_nc())
    nc = _NC_CACHE[0]
    try:
        res = run_bass_kernel_spmd(nc, in_maps, list(range(8)))
    except Exception:
        # transient NRT device faults have been observed once after crashed
        # runs; a clean retry in the same process recovers
        res = run_bass_kernel_spmd(nc, in_maps, list(range(8)))
    out = np.empty((NTOK, DOUT), np.float32)
    for c in range(8):
        i, j = c // TP, c % TP
        oc = res.results[c]["oT"]                      # (NRN_C, TOK_C)
        out[i * TOK_C:(i + 1) * TOK_C, j * NRN_C:(j + 1) * NRN_C] = oc.T
    return out.reshape(B, S, DOUT)
